# revision 1
# baseline (speedup 1.0000x reference)
"""AttentionBlock (GroupNorm + single-head spatial attention + residual) on 8 NeuronCores.

Data-parallel over batch: 16 batch elements -> 2 per core. Each core runs an
identical Bass/Tile program over its two batch elements, software-pipelined
(batch 1's loads/stats hide under batch 0's attention).

Per-batch dataflow (C=512 channels, N=H*W=1024 tokens; fp32 data, all large
matmuls run in float32r — the PE's fast replicated-fp32 mode, 4x the plain
fp32 matmul rate, ~7e-5 relative error on the final output):
  x [C, N] (channel per partition, 4 tiles of [128, 1024])
  GroupNorm: per-channel mean/E[x^2] via bn_stats/bn_aggr; 16-channel group
    combine + group->channel broadcast via tiny indicator-matrix matmuls on
    the PE (avoids cross-partition ops on DVE/GPSIMD).
  qkv:   q,k = WqkT.T @ h    (lhsT = WqkT [c, 1024], rhs = h)
         vT  = h.T @ WvT     (lhsT = h, rhs = WvT [c, 512]) -> [N, C] layout,
         so the attention@V matmul needs no transposes anywhere.
  scores: S^T[m, n] = k.T @ q computed directly in transposed layout
         (lhsT = k column block, rhs = q). exp applied on PSUM eviction by
         ScalarE; no max subtraction (scores are ~N(0,1), |S|max ~ 6).
  colsum: exp tiles are running-summed on DVE ([128, N] accumulator), then a
         single ones-vector PE pass reduces over partitions; the cs/rb
         matmuls are emitted mid-AV so the in-order PE stream never stalls
         on the accumulator.
  AV:    O^T[c, n] = vT.T @ A (lhsT = vT column block, rhs = A), normalized
         by 1/colsum (PE-broadcast row) during PSUM eviction.
  proj + residual: out = x + OwT.T @ O^T + bias_eff, fused into eviction.

qkv/out biases: q,k biases applied on eviction (per-partition). The v bias
commutes through softmax (rows sum to 1) and the out projection, so it is
folded on the host: bias_eff = out_w @ bv + out_b.

Infra notes: this walrus build allows ONE sync-wait per ISA instruction, so
_split_multi_waits() hoists extra waits onto same-engine NoOps. Every
producer feeding a float32r matmul (DMAs included) must write through a
float32r-typed view or the BIR verifier rejects the program.
"""

import math

import numpy as np

B, C, N = 16, 512, 1024
G = 32
EPS = 1e-5
NCORES = 8
BPC = B // NCORES  # batches per core
CT = C // 128      # channel tiles (4)
NT = N // 128      # token tiles (8)
KC = C // 128      # contraction chunks over channels (4)
KM = N // 128      # contraction chunks over tokens (8)
HALF = 512         # PSUM bank free size (fp32)
SCALE = 1.0 / math.sqrt(C)

# packed small-constant tile column layout [128, SC_COLS]:
#   0: ones column; 1..128: ones row (row 0); then per-chunk gnw/gnb/beff,
#   qkb, gfwd [128,32] blocks, gbwd [32,128] blocks (rows 0..31)
SC_ONEC = 0
SC_ONER = 1          # [0:1, SC_ONER:SC_ONER+128]
SC_GNW = 129         # +t
SC_GNB = 133
SC_BEFF = 137
SC_QKB = 141         # +mt (8)
SC_GFWD = 149        # +32*t, width 32
SC_GBWD = 277        # +128*t, rows 0..32, width 128
SC_COLS = 789

_CACHE = {}


def _build():
    import concourse.bass as bass
    import concourse.tile as tile
    from concourse import mybir
    from contextlib import ExitStack

    f32 = mybir.dt.float32
    Alu = mybir.AluOpType
    Act = mybir.ActivationFunctionType

    def r(ap):
        # float32r: same fp32 bytes, PE replication mode (1 cycle/row at N>=256
        # vs 4 for plain fp32)
        return ap.bitcast(mybir.dt.float32r)

    nc = bass.Bass("TRN2", target_bir_lowering=False)

    x_d = nc.dram_tensor("x", [BPC, CT, 128, N], f32, kind="ExternalInput")
    wqk_d = nc.dram_tensor("wqk", [128, KC, 1024], f32, kind="ExternalInput")
    wv_d = nc.dram_tensor("wv", [128, KC, 512], f32, kind="ExternalInput")
    ow_d = nc.dram_tensor("ow", [128, KC, 512], f32, kind="ExternalInput")
    smallc_d = nc.dram_tensor("smallc", [128, SC_COLS], f32, kind="ExternalInput")
    out_d = nc.dram_tensor("out", [BPC, CT, 128, N], f32, kind="ExternalOutput")

    with ExitStack() as ctx:
        # float32r views trip the low-precision accumulation guard; rounding
        # fp32 -> fp32r costs ~2^-17 relative, far inside our error budget.
        ctx.enter_context(nc.allow_low_precision("float32r PE fast path"))
        tc = ctx.enter_context(tile.TileContext(nc))
        consts = ctx.enter_context(tc.tile_pool(name="consts", bufs=1))
        xp = ctx.enter_context(tc.tile_pool(name="xp", bufs=2 * CT))
        hp = ctx.enter_context(tc.tile_pool(name="hp", bufs=CT))
        qp = ctx.enter_context(tc.tile_pool(name="qp", bufs=CT))
        kp = ctx.enter_context(tc.tile_pool(name="kp", bufs=CT))
        vp = ctx.enter_context(tc.tile_pool(name="vp", bufs=NT))
        ap_ = ctx.enter_context(tc.tile_pool(name="ap_", bufs=NT))
        op_ = ctx.enter_context(tc.tile_pool(name="op_", bufs=CT))
        rp = ctx.enter_context(tc.tile_pool(name="rp", bufs=1))
        outp = ctx.enter_context(tc.tile_pool(name="outp", bufs=2))
        smallp = ctx.enter_context(tc.tile_pool(name="smallp", bufs=2))
        pmm = ctx.enter_context(tc.tile_pool(name="pmm", bufs=6, space="PSUM"))
        psm = ctx.enter_context(tc.tile_pool(name="psm", bufs=2, space="PSUM"))

        # --- constants: one packed DMA for all small tensors, one DMA per
        # weight matrix (many small DMAs serialized ~625ns each on HWDGE and
        # starved the batch-0 x loads)
        smallc = consts.tile([128, SC_COLS], f32, tag="smallc", name="smallc")
        nc.sync.dma_start(out=r(smallc), in_=r(smallc_d[:, :]))
        ones_m = smallc[:, SC_ONEC:SC_ONEC + 1]
        ones_r = smallc[0:1, SC_ONER:SC_ONER + 128]
        gnw = [smallc[:, SC_GNW + t:SC_GNW + t + 1] for t in range(CT)]
        gnb = [smallc[:, SC_GNB + t:SC_GNB + t + 1] for t in range(CT)]
        beff = [smallc[:, SC_BEFF + t:SC_BEFF + t + 1] for t in range(CT)]
        qkb = [smallc[:, SC_QKB + t:SC_QKB + t + 1] for t in range(2 * CT)]
        gfwd = [
            smallc[:, SC_GFWD + G * t:SC_GFWD + G * (t + 1)].bitcast(f32)
            for t in range(CT)
        ]
        gbwd = [
            smallc[0:G, SC_GBWD + 128 * t:SC_GBWD + 128 * (t + 1)].bitcast(f32)
            for t in range(CT)
        ]
        wqkt = consts.tile([128, KC, 1024], f32, tag="wqkt", name="wqkt")
        wvt = consts.tile([128, KC, 512], f32, tag="wvt", name="wvt")
        owt = consts.tile([128, KC, 512], f32, tag="owt", name="owt")
        wqk = [wqkt[:, t, :] for t in range(KC)]
        wv = [wvt[:, t, :] for t in range(KC)]
        ow = [owt[:, t, :] for t in range(KC)]
        eps_t = consts.tile([G, 1], f32, tag="eps_t", name="eps_t")
        nc.vector.memset(eps_t, EPS)

        xt = {}
        ht = {}
        stt = {}
        qt = {}
        kt = {}
        vt = {}
        at = {}
        ot = {}
        rbs = {}
        accs = {}

        def emit_stats(b):
            # x loads + per-channel stats; st col0 = mean, col1 = E[x^2]
            xt[b] = []
            stt[b] = []
            for t in range(CT):
                x1 = xp.tile([128, N], f32, tag="x", name=f"x{b}_{t}")
                nc.sync.dma_start(out=x1, in_=x_d[b, t])
                xt[b].append(x1)
                st = smallp.tile([128, 2], f32, tag="st", name=f"st{b}_{t}")
                st6 = smallp.tile([128, 2, 6], f32, tag="st6", name=f"st6{b}_{t}")
                xv = x1.rearrange("p (s f) -> p s f", s=2)
                for s_ in range(2):
                    nc.vector.bn_stats(out=st6[:, s_, :], in_=xv[:, s_, :])
                nc.vector.bn_aggr(out=st, in_=st6)
                t0 = smallp.tile([128, 1], f32, tag="st0", name=f"st0{b}_{t}")
                nc.vector.tensor_mul(out=t0, in0=st[:, 0:1], in1=st[:, 0:1])
                nc.vector.tensor_add(out=st[:, 1:2], in0=st[:, 1:2], in1=t0)
                stt[b].append(st)

        def emit_gn_rest(b):
            # group combine (PE) for all tiles, then all stat chains, then all
            # broadcasts + applies -- keeps DVE from blocking slot releases
            ht[b] = []
            gsums = []
            for t in range(CT):
                gsum_ps = psm.tile([G, 2], f32, tag="gn", name=f"gsum{b}_{t}")
                nc.tensor.matmul(gsum_ps, gfwd[t], stt[b][t], start=True, stop=True)
                gsums.append(gsum_ps)
            gb2s = []
            for t in range(CT):
                gb2 = smallp.tile([G, 2], f32, tag=f"gb2{t}", name=f"gb2{b}_{t}")
                tmp = smallp.tile([G, 2], f32, tag=f"gtmp{t}", name=f"gtmp{b}_{t}")
                nc.vector.tensor_scalar_mul(
                    out=gb2[:, 0:1], in0=gsums[t][:, 0:1], scalar1=1.0 / 16.0
                )
                nc.vector.tensor_mul(
                    out=tmp[:, 0:1], in0=gb2[:, 0:1], in1=gb2[:, 0:1]
                )
                nc.vector.scalar_tensor_tensor(
                    out=tmp[:, 1:2], in0=gsums[t][:, 1:2], scalar=1.0 / 16.0,
                    in1=tmp[:, 0:1], op0=Alu.mult, op1=Alu.subtract,
                )
                nc.scalar.activation(
                    out=tmp[:, 0:1], in_=tmp[:, 1:2], func=Act.Sqrt, bias=eps_t
                )
                nc.vector.reciprocal(out=gb2[:, 1:2], in_=tmp[:, 0:1])
                gb2s.append(gb2)
            mcs = []
            for t in range(CT):
                mc_ps = psm.tile([128, 2], f32, tag="gn", name=f"mc{b}_{t}")
                nc.tensor.matmul(mc_ps, gbwd[t], gb2s[t], start=True, stop=True)
                mcs.append(mc_ps)
            for t in range(CT):
                a1 = smallp.tile([128, 1], f32, tag=f"ac{t}", name=f"ac{b}_{t}")
                t1 = smallp.tile([128, 1], f32, tag=f"tc{t}", name=f"tc{b}_{t}")
                nc.vector.tensor_mul(out=a1, in0=mcs[t][:, 1:2], in1=gnw[t])
                # t1 = mean_c*ac - gnb, applied as h = x*ac - t1
                nc.vector.scalar_tensor_tensor(
                    out=t1, in0=mcs[t][:, 0:1], scalar=a1, in1=gnb[t],
                    op0=Alu.mult, op1=Alu.subtract,
                )
                h1 = hp.tile([128, N], f32, tag="h", name=f"h{b}_{t}")
                nc.vector.tensor_scalar(
                    out=r(h1), in0=xt[b][t], scalar1=a1, scalar2=t1,
                    op0=Alu.mult, op1=Alu.subtract,
                )
                ht[b].append(h1)

        def emit_qkv(b):
            qt[b] = []
            kt[b] = []
            for mt in range(2 * CT):
                dest = qp if mt < CT else kp
                qk1 = dest.tile(
                    [128, N], f32, tag="q" if mt < CT else "k",
                    name=f"{'q' if mt < CT else 'k'}{b}_{mt % CT}",
                )
                # h-inner order: both halves reuse each stationary operand, so
                # the PE issues one LDWEIGHTS per two matmuls
                pss = [
                    pmm.tile([128, HALF], f32, tag="mm", name=f"psqk{b}_{mt}_{h}")
                    for h in range(2)
                ]
                for c in range(KC):
                    for h in range(2):
                        nc.tensor.matmul(
                            pss[h], r(wqk[c][:, mt * 128:(mt + 1) * 128]),
                            r(ht[b][c][:, h * HALF:(h + 1) * HALF]),
                            start=(c == 0), stop=(c == KC - 1),
                            skip_group_check=True,
                        )
                for h in range(2):
                    nc.scalar.activation(
                        out=r(qk1[:, h * HALF:(h + 1) * HALF]), in_=pss[h],
                        func=Act.Identity, bias=qkb[mt],
                    )
                (qt[b] if mt < CT else kt[b]).append(qk1)
            vt[b] = []
            for nt in range(NT):
                v1 = vp.tile([128, C], f32, tag="v", name=f"v{b}_{nt}")
                ps = pmm.tile([128, HALF], f32, tag="mm", name=f"psv{b}_{nt}")
                for c in range(KC):
                    nc.tensor.matmul(
                        ps, r(ht[b][c][:, nt * 128:(nt + 1) * 128]), r(wv[c]),
                        start=(c == 0), stop=(c == KC - 1),
                    )
                nc.scalar.activation(out=r(v1), in_=ps, func=Act.Identity)
                vt[b].append(v1)

        def emit_scores(b):
            at[b] = []
            acc = rp.tile([128, N], f32, tag="acc", name=f"acc{b}")
            for mt in range(NT):
                a1 = ap_.tile([128, N], f32, tag="a", name=f"a{b}_{mt}")
                pss = [
                    pmm.tile([128, HALF], f32, tag="mm", name=f"pss{b}_{mt}_{h}")
                    for h in range(2)
                ]
                for c in range(KC):
                    for h in range(2):
                        nc.tensor.matmul(
                            pss[h], r(kt[b][c][:, mt * 128:(mt + 1) * 128]),
                            r(qt[b][c][:, h * HALF:(h + 1) * HALF]),
                            start=(c == 0), stop=(c == KC - 1),
                            skip_group_check=True,
                        )
                for h in range(2):
                    nc.scalar.activation(
                        out=r(a1[:, h * HALF:(h + 1) * HALF]), in_=pss[h],
                        func=Act.Exp, scale=SCALE,
                    )
                at[b].append(a1)
                # running partial-sum of exp tiles on DVE (cheap vs 16 extra
                # full PE streaming passes)
                if mt == 0:
                    nc.vector.tensor_copy(out=r(acc), in_=a1)
                else:
                    nc.vector.tensor_add(out=r(acc), in0=acc, in1=a1)
            accs[b] = acc

        def emit_rb(b):
            # partition-sum of the accumulator: one PE pass
            cs_ps = [
                psm.tile([1, HALF], f32, tag="gn", name=f"cs{b}_{h}")
                for h in range(2)
            ]
            for h in range(2):
                nc.tensor.matmul(
                    cs_ps[h], r(ones_m),
                    r(accs[b][:, h * HALF:(h + 1) * HALF]),
                    start=True, stop=True,
                )
            # broadcast the sums to 128 partitions via PE, then reciprocal at
            # full 128-lane width (a [1, N] reciprocal runs on a single DVE
            # lane and was the batch-boundary critical path)
            srow = rp.tile([1, N], f32, tag="srow", name=f"srow{b}")
            for h in range(2):
                nc.scalar.activation(
                    out=r(srow[:, h * HALF:(h + 1) * HALF]), in_=cs_ps[h],
                    func=Act.Identity,
                )
            rb = rp.tile([128, N], f32, tag="rb", name=f"rb{b}")
            for h in range(2):
                ps = pmm.tile([128, HALF], f32, tag="mm", name=f"psrb{b}_{h}")
                nc.tensor.matmul(
                    ps, r(ones_r), r(srow[:, h * HALF:(h + 1) * HALF]),
                    start=True, stop=True,
                )
                nc.vector.reciprocal(out=rb[:, h * HALF:(h + 1) * HALF], in_=ps)
            rbs[b] = rb

        av_ps = {}

        def emit_av_mm(b, cts):
            for ct_ in cts:
                pss = [
                    pmm.tile([128, HALF], f32, tag="mm", name=f"pso{b}_{ct_}_{h}")
                    for h in range(2)
                ]
                for m in range(KM):
                    for h in range(2):
                        nc.tensor.matmul(
                            pss[h], r(vt[b][m][:, ct_ * 128:(ct_ + 1) * 128]),
                            r(at[b][m][:, h * HALF:(h + 1) * HALF]),
                            start=(m == 0), stop=(m == KM - 1),
                            skip_group_check=True,
                        )
                for h in range(2):
                    av_ps[(b, ct_, h)] = pss[h]

        def emit_av_evict(b, cts):
            for ct_ in cts:
                o1 = op_.tile([128, N], f32, tag="o", name=f"o{b}_{ct_}")
                for h in range(2):
                    nc.vector.tensor_mul(
                        out=r(o1[:, h * HALF:(h + 1) * HALF]),
                        in0=av_ps.pop((b, ct_, h)),
                        in1=rbs[b][:, h * HALF:(h + 1) * HALF],
                    )
                ot[b].append(o1)

        def emit_proj(b):
            for t in range(CT):
                f1 = outp.tile([128, N], f32, tag="f", name=f"f{b}_{t}")
                pss = [
                    pmm.tile([128, HALF], f32, tag="mm", name=f"psp{b}_{t}_{h}")
                    for h in range(2)
                ]
                for c in range(KC):
                    for h in range(2):
                        nc.tensor.matmul(
                            pss[h], r(ow[c][:, t * 128:(t + 1) * 128]),
                            r(ot[b][c][:, h * HALF:(h + 1) * HALF]),
                            start=(c == 0), stop=(c == KC - 1),
                            skip_group_check=True,
                        )
                for h in range(2):
                    nc.vector.scalar_tensor_tensor(
                        out=f1[:, h * HALF:(h + 1) * HALF], in0=pss[h],
                        scalar=beff[t],
                        in1=xt[b][t][:, h * HALF:(h + 1) * HALF],
                        op0=Alu.add, op1=Alu.add,
                    )
                for h in range(2):
                    nc.sync.dma_start(
                        out=out_d[b, t, :, h * HALF:(h + 1) * HALF],
                        in_=f1[:, h * HALF:(h + 1) * HALF],
                    )

        # software-pipelined emission: batch 1's x loads + stats hide under
        # batch 0's attention phases
        emit_stats(0)
        emit_gn_rest(0)
        nc.sync.dma_start(out=r(wqkt), in_=r(wqk_d[:, :, :]))
        nc.sync.dma_start(out=r(wvt), in_=r(wv_d[:, :, :]))
        emit_qkv(0)
        emit_stats(1)
        nc.sync.dma_start(out=r(owt), in_=r(ow_d[:, :, :]))
        emit_scores(0)
        ot[0] = []
        emit_av_mm(0, [0, 1])
        emit_rb(0)
        emit_av_evict(0, [0, 1])
        emit_av_mm(0, [2, 3])
        emit_av_evict(0, [2, 3])
        emit_gn_rest(1)
        emit_proj(0)
        emit_qkv(1)
        emit_scores(1)
        ot[1] = []
        emit_av_mm(1, [0, 1])
        emit_rb(1)
        emit_av_evict(1, [0, 1])
        emit_av_mm(1, [2, 3])
        emit_av_evict(1, [2, 3])
        emit_proj(1)

    _split_multi_waits(nc)
    return nc


def _split_multi_waits(nc):
    """This neuronxcc walrus supports one sync-wait per ISA instruction.

    Tile emits instructions with several waits; hoist all but the last onto
    same-engine NoOps inserted immediately before (engine sequencers execute
    waits in order, so this is semantically identical).
    """
    from concourse import mybir

    n = 0
    for f in nc.m.functions:
        for bb in f.blocks:
            insts = bb.instructions
            out = []
            for inst in insts:
                si = inst.sync_info
                if si is not None and si.on_wait and len(si.on_wait) > 1:
                    waits = list(si.on_wait)
                    for w in waits[:-1]:
                        nop = mybir.InstNoOp(name=f"WSPLIT-{n}", ins=[], outs=[])
                        n += 1
                        nop.engine = inst.engine
                        nop.sync_info = mybir.SyncInfo(on_wait=[w], on_update=[])
                        out.append(nop)
                    inst.sync_info = mybir.SyncInfo(
                        on_wait=[waits[-1]], on_update=list(si.on_update or [])
                    )
                out.append(inst)
            if n:
                bb.instructions = out
    return nc


def _prep_consts(qkv_w, qkv_b, out_w, out_b, gn_w, gn_b):
    f = np.float32
    # lhsT chunk layouts: dev tensor [128, KC, Ncols]; [p, t, o] = W.T[128t+p, o]
    wqk = np.ascontiguousarray(
        qkv_w[: 2 * C].T.reshape(KC, 128, 2 * C).transpose(1, 0, 2), dtype=f)
    wv = np.ascontiguousarray(
        qkv_w[2 * C:].T.reshape(KC, 128, C).transpose(1, 0, 2), dtype=f)
    ow = np.ascontiguousarray(
        out_w.T.reshape(KC, 128, C).transpose(1, 0, 2), dtype=f)
    smallc = np.zeros((128, SC_COLS), dtype=f)
    smallc[:, SC_ONEC] = 1.0
    smallc[0, SC_ONER:SC_ONER + 128] = 1.0
    for t in range(CT):
        cs = slice(128 * t, 128 * (t + 1))
        smallc[:, SC_GNW + t] = gn_w[cs]
        smallc[:, SC_GNB + t] = gn_b[cs]
    beff = out_w @ qkv_b[2 * C:] + out_b
    for t in range(CT):
        smallc[:, SC_BEFF + t] = beff[128 * t:128 * (t + 1)]
    for mt in range(2 * CT):
        smallc[:, SC_QKB + mt] = qkv_b[128 * mt:128 * (mt + 1)]
    for t in range(CT):
        for p_ in range(128):
            smallc[p_, SC_GFWD + G * t + (128 * t + p_) // 16] = 1.0
            smallc[(128 * t + p_) // 16, SC_GBWD + 128 * t + p_] = 1.0
    return dict(wqk=wqk, wv=wv, ow=ow, smallc=smallc)


def kernel(x, gn_w, gn_b, qkv_w, qkv_b, out_w, out_b):
    from concourse.bass_utils import run_bass_kernel_spmd

    x = np.ascontiguousarray(np.asarray(x, dtype=np.float32))
    consts = _prep_consts(
        np.asarray(qkv_w, np.float32), np.asarray(qkv_b, np.float32),
        np.asarray(out_w, np.float32), np.asarray(out_b, np.float32),
        np.asarray(gn_w, np.float32), np.asarray(gn_b, np.float32),
    )
    xr = x.reshape(NCORES, BPC, CT, 128, N)
    in_maps = [dict(x=np.ascontiguousarray(xr[i]), **consts) for i in range(NCORES)]

    if "nc" not in _CACHE:
        _CACHE["nc"] = _build()
    res = run_bass_kernel_spmd(
        _CACHE["nc"], in_maps, core_ids=list(range(NCORES)),
        trace=_CACHE.get("trace", False),
    )
    _CACHE["last"] = res
    out = np.stack([r["out"] for r in res.results])  # [8, BPC, CT, 128, N]
    return out.reshape(B, C, 32, 32)



# revision 2
# speedup vs baseline: 1.0460x; 1.0460x over previous
"""AttentionBlock (GroupNorm + single-head spatial attention + residual) on 8
NeuronCores — fp8 DoubleRow edition.

Data-parallel over batch: 16 batch elements -> 2 per core, software-pipelined.

All five big matmuls (qkv, v, scores, attn@V, out-proj) run as fp8e4m3
DoubleRow matmuls (2 fp8 weights per PE cell -> K=256 per instruction).
Scaling scheme keeps every fp8 operand in the healthy e4m3 range and folds
all compensation into existing free scale slots:
  - weights scaled x16 on host (their natural sigma ~1/sqrt(C)=0.044 sits in
    the denormal range; x16 moves it to ~0.7)
  - q8,k8 = 16*(q,k) (sigma ~16); scores psum = 256*S -> exp scale SCALE/256
  - A8 = exp(S - ln16): keeps exp(S) <= ~31 under the 448 fp8 max; the
    shift cancels exactly in the softmax normalization
  - colsum of A8 via a DoubleRow ones-matmul whose [2,128] ones lhsT
    replicates the sum across all 128 output partitions -> rb = 1/colsum via
    one DVE reciprocal, no separate broadcast
  - v8 = 16*v -> O' = psum_av * rb = 16*O (sigma ~0.8); proj psum = 256*proj
    -> final eviction multiplies by 1/256 and adds the residual in one DVE op.

Engine split: PE does all matmuls + GN group-combine; ACT evicts q/k (+bias)
and exp(A) (+ batch-1 v); DVE evicts batch-0 v, AV and proj, and runs the
packed GN chain; GN stats run as bn_stats on DVE for batch 0 (latency) and
as accum_out sum/sum-sq passes on the otherwise-idle Pool for batch 1; the
GN scale/shift (h8) is spread across ACT/DVE/Pool for batch 0 and parked on
Pool for batch 1 (hidden under scores(0) PE work).

Infra notes: this walrus build allows ONE sync-wait per ISA instruction, so
_split_multi_waits() hoists extra waits onto same-engine NoOps.
"""

import math

import numpy as np

B, C, N = 16, 512, 1024
G = 32
EPS = 1e-5
NCORES = 8
BPC = B // NCORES  # batches per core
CT = C // 128      # channel tiles (4)
NT = N // 128      # token tiles (8)
KC = C // 128      # contraction chunks over channels (4)
KM = N // 128      # contraction chunks over tokens (8)
HALF = 512
WS = 16.0          # host weight scale
SCALE = 1.0 / math.sqrt(C)
LN16 = 4.0 * math.log(2.0)

# packed fp32 small-constant tiles
# smallc [128, 16]: gnw(4) gnb(4) qkb16(8) — tiny, DMA'd first
SC_GNW = 0
SC_GNB = 4
SC_QKB = 8
SC_COLS = 16
# gind [128, GI_COLS]: GN indicator matmul operands + rank-1 bias operands
GI_GFWD = 0          # +32*t, width 32
GI_GBWD = 128        # +128*t, rows 0..31, width 128
GI_BEFFR = 640       # row0: 256*beff, 4 blocks of 128
GI_ONER = 1152       # row0: ones, 512 wide
GI_COLS = 1664

_CACHE = {}


def _build(with_beff):
    import concourse.bass as bass
    import concourse.tile as tile
    from concourse import mybir
    from contextlib import ExitStack

    f32 = mybir.dt.float32
    f8 = mybir.dt.float8e4
    Alu = mybir.AluOpType
    Act = mybir.ActivationFunctionType
    DR = mybir.MatmulPerfMode.DoubleRow

    nc = bass.Bass("TRN2", target_bir_lowering=False)

    def r(ap):
        return ap.bitcast(mybir.dt.float32r)

    x_d = nc.dram_tensor("x", [BPC, CT, 128, N], f32, kind="ExternalInput")
    x8_d = nc.dram_tensor("x8", [BPC, CT, 128, N], mybir.dt.bfloat16,
                          kind="ExternalInput")
    wqk_d = nc.dram_tensor("wqk", [128, KC, 2 * C], f8, kind="ExternalInput")
    wv_d = nc.dram_tensor("wv", [128, KC, C], f8, kind="ExternalInput")
    ow_d = nc.dram_tensor("ow", [128, KC, C], f8, kind="ExternalInput")
    smallc_d = nc.dram_tensor("smallc", [128, SC_COLS], f32, kind="ExternalInput")
    gind_d = nc.dram_tensor("gind", [128, GI_COLS], f32, kind="ExternalInput")
    id_d = nc.dram_tensor("id256", [128, 128], f32, kind="ExternalInput")
    out_d = nc.dram_tensor("out", [BPC, CT, 128, N], f32, kind="ExternalOutput")

    with ExitStack() as ctx:
        ctx.enter_context(nc.allow_low_precision("fp8 DoubleRow path"))
        tc = ctx.enter_context(tile.TileContext(nc))
        consts = ctx.enter_context(tc.tile_pool(name="consts", bufs=1))
        xp = ctx.enter_context(tc.tile_pool(name="xp", bufs=2 * CT))
        xp8 = ctx.enter_context(tc.tile_pool(name="xp8", bufs=2 * CT))
        hp = ctx.enter_context(tc.tile_pool(name="hp", bufs=2))
        qp = ctx.enter_context(tc.tile_pool(name="qp", bufs=2))
        kp = ctx.enter_context(tc.tile_pool(name="kp", bufs=2))
        vp = ctx.enter_context(tc.tile_pool(name="vp", bufs=2))
        ap_ = ctx.enter_context(tc.tile_pool(name="ap_", bufs=2))
        op_ = ctx.enter_context(tc.tile_pool(name="op_", bufs=2))
        rp = ctx.enter_context(tc.tile_pool(name="rp", bufs=2))
        outp = ctx.enter_context(tc.tile_pool(name="outp", bufs=8))
        smallp = ctx.enter_context(tc.tile_pool(name="smallp", bufs=2))
        scrp = ctx.enter_context(tc.tile_pool(name="scrp", bufs=1))
        pmm = ctx.enter_context(tc.tile_pool(name="pmm", bufs=3, space="PSUM"))
        pcs = ctx.enter_context(tc.tile_pool(name="pcs", bufs=1, space="PSUM"))

        # --- const tiles (DMAs emitted inside the schedule below)
        smallc = consts.tile([128, SC_COLS], f32, tag="smallc", name="smallc")
        gind = consts.tile([128, GI_COLS], f32, tag="gind", name="gind")
        gnw4 = smallc[:, SC_GNW:SC_GNW + 4]
        gnb4 = smallc[:, SC_GNB:SC_GNB + 4]
        qkb = [smallc[:, SC_QKB + m:SC_QKB + m + 1] for m in range(2 * CT)]
        gfwd = [gind[:, GI_GFWD + G * t:GI_GFWD + G * (t + 1)] for t in range(CT)]
        gbwd = [gind[0:G, GI_GBWD + 128 * t:GI_GBWD + 128 * (t + 1)]
                for t in range(CT)]
        id256 = consts.tile([128, 128], f32, tag="id256", name="id256")
        wqk8 = consts.tile([128, KC, 2 * C], f8, tag="wqk8", name="wqk8")
        wv8 = consts.tile([128, KC, C], f8, tag="wv8", name="wv8")
        ow8 = consts.tile([128, KC, C], f8, tag="ow8", name="ow8")
        eps_t = consts.tile([G, 1], f32, tag="eps_t", name="eps_t")
        nc.vector.memset(eps_t, EPS)
        nl16 = consts.tile([128, 1], f32, tag="nl16", name="nl16")
        nc.vector.memset(nl16, -LN16)
        ones8 = consts.tile([128, 2, 128], f8, tag="ones8", name="ones8")
        nc.vector.memset(ones8, 1.0)
        scr = scrp.tile([128, N], f32, tag="scr", name="scr")  # Pool stats junk

        xt = {}
        x8t = {}
        ht = {}
        st8s = {}
        qt = {}
        kt = {}
        vt = {}
        at = {}
        ot = {}
        rbs = {}

        bf16 = mybir.dt.bfloat16

        def emit_x8loads(b):
            # bf16 copy of x: feeds GN stats + the h8 apply (h8 is fp8 anyway
            # so bf16 input precision is irrelevant); half the DMA bytes of
            # fp32 so the startup-critical stats path unblocks sooner
            x8t[b] = []
            for t in range(CT):
                x1 = xp8.tile([128, N], bf16, tag="x8", name=f"x8_{b}_{t}")
                nc.sync.dma_start(out=x1, in_=x8_d[b, t])
                x8t[b].append(x1)

        def emit_xloads(b, as_r=False):
            # fp32 x, only needed for the late residual-add; batch 1 is
            # consumed by an fp32r matmul so its DMA writes the r-typed view
            xt[b] = []
            for t in range(CT):
                x1 = xp.tile([128, N], f32, tag="x", name=f"x{b}_{t}")
                if as_r:
                    nc.sync.dma_start(out=r(x1), in_=r(x_d[b, t]))
                else:
                    nc.sync.dma_start(out=x1, in_=x_d[b, t])
                xt[b].append(x1)

        def emit_stats_dve(b):
            # bn_stats/bn_aggr -> st8 pairs (mean, var); per-tile var ->
            # E[x^2] fixup so it pipelines under the next tile's bn_stats
            st8 = smallp.tile([128, 8], f32, tag="st8", name=f"st8_{b}")
            st8s[b] = st8
            for t in range(CT):
                st6 = smallp.tile([128, 2, 6], f32, tag=f"st6{t}", name=f"st6{b}_{t}")
                xv = x8t[b][t].rearrange("p (s f) -> p s f", s=2)
                for s_ in range(2):
                    nc.vector.bn_stats(out=st6[:, s_, :], in_=xv[:, s_, :])
                nc.vector.bn_aggr(out=st8[:, 2 * t:2 * t + 2], in_=st6)
                msq = smallp.tile([128, 1], f32, tag=f"msq{t}", name=f"msq{b}_{t}")
                nc.vector.tensor_mul(
                    out=msq, in0=st8[:, 2 * t:2 * t + 1], in1=st8[:, 2 * t:2 * t + 1])
                nc.vector.tensor_add(
                    out=st8[:, 2 * t + 1:2 * t + 2],
                    in0=st8[:, 2 * t + 1:2 * t + 2], in1=msq)

        def emit_stats_pool(b):
            # raw sum / sum-sq per channel on the otherwise-idle Pool engine
            # via accum_out; the 1/N scaling folds into the group-combine
            # chain constants (st_scale)
            st8 = smallp.tile([128, 8], f32, tag="st8", name=f"st8_{b}")
            st8s[b] = st8
            for t in range(CT):
                nc.gpsimd.tensor_scalar(
                    out=scr, in0=x8t[b][t], scalar1=1.0, scalar2=0.0,
                    op0=Alu.mult, op1=Alu.add,
                    accum_out=st8[:, 2 * t:2 * t + 1],
                )
                nc.gpsimd.scalar_tensor_tensor(
                    out=scr, in0=x8t[b][t], scalar=1.0, in1=x8t[b][t],
                    op0=Alu.mult, op1=Alu.mult,
                    accum_out=st8[:, 2 * t + 1:2 * t + 2],
                )

        def emit_gn_chain(b, st_scale):
            # group combine (PE indicator matmuls into a [G,8] psum slice) +
            # packed group-stat chain -> meanInv8 (mean at even, 1/sd at odd)
            st8 = st8s[b]
            gn_ps = pmm.tile([128, N], f32, tag="mm", name=f"gnps{b}")
            gsum8 = gn_ps[0:G, 0:8]
            for t in range(CT):
                nc.tensor.matmul(
                    gsum8[:, 2 * t:2 * t + 2], gfwd[t], st8[:, 2 * t:2 * t + 2],
                    start=True, stop=True, skip_group_check=True,
                )
            gsv = gsum8.rearrange("p (t s) -> p t s", s=2)
            meanInv8 = smallp.tile([G, 8], f32, tag="mi8", name=f"mi8_{b}")
            miv = meanInv8.rearrange("p (t s) -> p t s", s=2)
            nc.vector.tensor_scalar_mul(
                out=miv[:, :, 0:1], in0=gsv[:, :, 0:1], scalar1=st_scale / 16.0)
            msq2 = smallp.tile([G, 4], f32, tag="msq2", name=f"msq2_{b}")
            nc.vector.tensor_mul(out=msq2, in0=miv[:, :, 0:1], in1=miv[:, :, 0:1])
            var4 = smallp.tile([G, 4], f32, tag="var4", name=f"var4_{b}")
            nc.vector.scalar_tensor_tensor(
                out=var4, in0=gsv[:, :, 1:2], scalar=st_scale / 16.0, in1=msq2,
                op0=Alu.mult, op1=Alu.subtract,
            )
            sd4 = smallp.tile([G, 4], f32, tag="sd4", name=f"sd4_{b}")
            nc.scalar.activation(out=sd4, in_=var4, func=Act.Sqrt, bias=eps_t)
            nc.vector.reciprocal(out=miv[:, :, 1:2], in_=sd4)
            return meanInv8

        def emit_gn_apply(b, meanInv8, h_engines):
            # broadcast group mean/inv-sd to channels (PE), then per-channel
            # h8 = x*a1 + t1n on the engines named in h_engines
            mc_ps = pmm.tile([128, N], f32, tag="mm", name=f"mcps{b}")
            mc8 = mc_ps[:, 0:8]
            for t in range(CT):
                nc.tensor.matmul(
                    mc8[:, 2 * t:2 * t + 2], gbwd[t], meanInv8[:, 2 * t:2 * t + 2],
                    start=True, stop=True, skip_group_check=True,
                )
            mcv = mc8.rearrange("p (t s) -> p t s", s=2)
            a1_4 = smallp.tile([128, 4], f32, tag="a14", name=f"a14_{b}")
            nc.vector.tensor_mul(out=a1_4, in0=mcv[:, :, 1:2], in1=gnw4)
            tmp4 = smallp.tile([128, 4], f32, tag="tmp4", name=f"tmp4_{b}")
            nc.vector.tensor_mul(out=tmp4, in0=mcv[:, :, 0:1], in1=a1_4)
            t1n4 = smallp.tile([128, 4], f32, tag="t1n4", name=f"t1n4_{b}")
            nc.vector.tensor_sub(out=t1n4, in0=gnb4, in1=tmp4)
            h8 = hp.tile([128, KC, N], f8, tag="h8", name=f"h8_{b}")
            for t in range(CT):
                eng = h_engines[t]
                if eng == "act":
                    nc.scalar.activation(
                        out=h8[:, t, :], in_=x8t[b][t], func=Act.Identity,
                        scale=a1_4[:, t:t + 1], bias=t1n4[:, t:t + 1],
                    )
                else:
                    e = nc.vector if eng == "dve" else nc.gpsimd
                    e.tensor_scalar(
                        out=h8[:, t, :], in0=x8t[b][t],
                        scalar1=a1_4[:, t:t + 1], scalar2=t1n4[:, t:t + 1],
                        op0=Alu.mult, op1=Alu.add,
                    )
            ht[b] = h8

        def emit_qkv(b, v_eng):
            # interleave ACT-drained (q/k) and v psum groups so both
            # eviction engines stay busy
            h8 = ht[b]
            q8 = qp.tile([128, KC, N], f8, tag="q8", name=f"q8_{b}")
            k8 = kp.tile([128, KC, N], f8, tag="k8", name=f"k8_{b}")
            v8 = vp.tile([128, NT, HALF], f8, tag="v8", name=f"v8_{b}")
            qt[b], kt[b], vt[b] = q8, k8, v8

            def qk_group(mt):
                ps = pmm.tile([128, N], f32, tag="mm", name=f"psqk{b}_{mt}")
                for c in range(0, KC, 2):
                    for h in range(2):
                        nc.tensor.matmul(
                            ps[:, h * HALF:(h + 1) * HALF],
                            wqk8[:, c:c + 2, mt * 128:(mt + 1) * 128],
                            h8[:, c:c + 2, h * HALF:(h + 1) * HALF],
                            start=(c == 0), stop=(c == KC - 2),
                            perf_mode=DR, skip_group_check=True,
                        )
                dest = q8 if mt < CT else k8
                nc.scalar.activation(
                    out=dest[:, mt % CT, :], in_=ps, func=Act.Identity,
                    bias=qkb[mt],
                )

            def v_group(nt):
                ps = pmm.tile([128, N], f32, tag="mm", name=f"psv{b}_{nt}")
                for j in range(2):
                    for c in range(0, KC, 2):
                        nc.tensor.matmul(
                            ps[:, j * HALF:(j + 1) * HALF],
                            h8[:, c:c + 2, (nt + j) * 128:(nt + j + 1) * 128],
                            wv8[:, c:c + 2, :],
                            start=(c == 0), stop=(c == KC - 2),
                            perf_mode=DR, skip_group_check=True,
                        )
                if v_eng == "act":
                    nc.scalar.copy(out=v8[:, nt:nt + 2, :], in_=ps)
                else:
                    nc.vector.tensor_copy(out=v8[:, nt:nt + 2, :], in_=ps)

            for g in range(4):
                qk_group(g)
                qk_group(4 + g)
                v_group(2 * g)

        css = {}
        avheld = {}

        def emit_scores(b, mts, hold_av0=False):
            # scores + exp eviction; after each odd mt, the colsum chunk for
            # (mt-1, mt) is accumulated into the held cs psum via a DoubleRow
            # ones-matmul (its [2,128] ones lhsT replicates the sum across
            # all 128 output partitions, so rb needs no separate broadcast).
            # With hold_av0, the attn@V chunk for output channels 0..127 also
            # accumulates early into a held psum, shortening the tail.
            q8, k8 = qt[b], kt[b]
            if b not in at:
                at[b] = ap_.tile([128, KM, N], f8, tag="a8", name=f"a8_{b}")
                css[b] = pcs.tile([128, N], f32, tag="cs", name=f"cs{b}")
            a8 = at[b]
            cs = css[b]
            for mt in mts:
                ps = pmm.tile([128, N], f32, tag="mm", name=f"pss{b}_{mt}")
                for c in range(0, KC, 2):
                    for h in range(2):
                        nc.tensor.matmul(
                            ps[:, h * HALF:(h + 1) * HALF],
                            k8[:, c:c + 2, mt * 128:(mt + 1) * 128],
                            q8[:, c:c + 2, h * HALF:(h + 1) * HALF],
                            start=(c == 0), stop=(c == KC - 2),
                            perf_mode=DR, skip_group_check=True,
                        )
                nc.scalar.activation(
                    out=a8[:, mt, :], in_=ps, func=Act.Exp,
                    scale=SCALE / (WS * WS), bias=nl16,
                )
                if mt % 2 == 1:
                    m = mt - 1
                    for h in range(2):
                        nc.tensor.matmul(
                            cs[:, h * HALF:(h + 1) * HALF],
                            ones8[:, :, :],
                            a8[:, m:m + 2, h * HALF:(h + 1) * HALF],
                            start=(m == 0), stop=(m == KM - 2),
                            perf_mode=DR, skip_group_check=True,
                        )
                    if b in avheld:
                        for h in range(2):
                            nc.tensor.matmul(
                                avheld[b][:, h * HALF:(h + 1) * HALF],
                                vt[b][:, m:m + 2, 0:128],
                                a8[:, m:m + 2, h * HALF:(h + 1) * HALF],
                                start=(m == 0), stop=(m == KM - 2),
                                perf_mode=DR, skip_group_check=True,
                            )

        def emit_rb(b):
            rb = rp.tile([128, N], f32, tag="rb", name=f"rb{b}")
            nc.vector.reciprocal(out=rb, in_=css[b])
            rbs[b] = rb

        def emit_av(b, cts, half_evict=False):
            # half_evict: evict all four h0 halves first so the proj h0
            # matmuls can start ~2.5us earlier in the tail
            a8, v8 = at[b], vt[b]
            if b not in ot:
                ot[b] = op_.tile([128, KC, N], f8, tag="o8", name=f"o8_{b}")
            o8 = ot[b]
            pss = {}
            for ct_ in cts:
                ps = pmm.tile([128, N], f32, tag="mm", name=f"pso{b}_{ct_}")
                pss[ct_] = ps
                for m in range(0, KM, 2):
                    for h in range(2):
                        nc.tensor.matmul(
                            ps[:, h * HALF:(h + 1) * HALF],
                            v8[:, m:m + 2, ct_ * 128:(ct_ + 1) * 128],
                            a8[:, m:m + 2, h * HALF:(h + 1) * HALF],
                            start=(m == 0), stop=(m == KM - 2),
                            perf_mode=DR, skip_group_check=True,
                        )
                if half_evict:
                    nc.vector.tensor_mul(
                        out=o8[:, ct_, 0:HALF], in0=ps[:, 0:HALF],
                        in1=rbs[b][:, 0:HALF])
                else:
                    nc.vector.tensor_mul(out=o8[:, ct_, :], in0=ps, in1=rbs[b])
            if half_evict:
                for ct_ in cts:
                    nc.vector.tensor_mul(
                        out=o8[:, ct_, HALF:N], in0=pss[ct_][:, HALF:N],
                        in1=rbs[b][:, HALF:N])

        def emit_proj(b, ts_, res_pe=False):
            # res_pe: add the residual on the PE via a 256*I fp32r matmul so
            # the eviction is a plain ACT identity (frees DVE in the tail)
            o8 = ot[b]
            for t in ts_:
                ps = pmm.tile([128, N], f32, tag="mm", name=f"psp{b}_{t}")
                for c in range(0, KC, 2):
                    for h in range(2):
                        nc.tensor.matmul(
                            ps[:, h * HALF:(h + 1) * HALF],
                            ow8[:, c:c + 2, t * 128:(t + 1) * 128],
                            o8[:, c:c + 2, h * HALF:(h + 1) * HALF],
                            start=(c == 0),
                            stop=(c == KC - 2) and not with_beff and not res_pe,
                            perf_mode=DR, skip_group_check=True,
                        )
                if with_beff:
                    # rank-1 bias add: psum += (256*beff) x ones_row
                    for h in range(2):
                        nc.tensor.matmul(
                            ps[:, h * HALF:(h + 1) * HALF],
                            gind[0:1, GI_BEFFR + 128 * t:GI_BEFFR + 128 * (t + 1)],
                            gind[0:1, GI_ONER:GI_ONER + HALF],
                            start=False, stop=(h == 1) and not res_pe,
                            skip_group_check=True,
                        )
                f1 = outp.tile([128, N], f32, tag="f", name=f"f{b}_{t}")
                if res_pe:
                    for h in range(2):
                        nc.tensor.matmul(
                            ps[:, h * HALF:(h + 1) * HALF],
                            r(id256),
                            r(xt[b][t][:, h * HALF:(h + 1) * HALF]),
                            start=False, stop=(h == 1), skip_group_check=True,
                        )
                    nc.scalar.activation(
                        out=f1, in_=ps, func=Act.Identity,
                        scale=1.0 / (WS * WS))
                else:
                    nc.vector.scalar_tensor_tensor(
                        out=f1, in0=ps, scalar=1.0 / (WS * WS), in1=xt[b][t],
                        op0=Alu.mult, op1=Alu.add,
                    )
                for h in range(2):
                    nc.sync.dma_start(
                        out=out_d[b, t, :, h * HALF:(h + 1) * HALF],
                        in_=f1[:, h * HALF:(h + 1) * HALF])

        def emit_proj_halved(b):
            # tail variant: per n-half proj with PE residual-add and ACT
            # eviction, so stores start while the h1 halves still compute
            o8 = ot[b]
            f1s = {}
            for t in range(CT):
                f1s[t] = outp.tile([128, N], f32, tag="f", name=f"f{b}_{t}")
            for h in range(2):
                pss = {}
                for t in range(CT):
                    ps = pmm.tile([128, HALF], f32, tag="mm",
                                  name=f"psp{b}_{t}_{h}")
                    pss[t] = ps
                    for c in range(0, KC, 2):
                        nc.tensor.matmul(
                            ps, ow8[:, c:c + 2, t * 128:(t + 1) * 128],
                            o8[:, c:c + 2, h * HALF:(h + 1) * HALF],
                            start=(c == 0), stop=False,
                            perf_mode=DR, skip_group_check=True,
                        )
                    if with_beff:
                        nc.tensor.matmul(
                            ps,
                            gind[0:1, GI_BEFFR + 128 * t:GI_BEFFR + 128 * (t + 1)],
                            gind[0:1, GI_ONER:GI_ONER + HALF],
                            start=False, stop=False, skip_group_check=True,
                        )
                    nc.tensor.matmul(
                        ps, r(id256),
                        r(xt[b][t][:, h * HALF:(h + 1) * HALF]),
                        start=False, stop=True, skip_group_check=True,
                    )
                for t in range(CT):
                    fh = f1s[t][:, h * HALF:(h + 1) * HALF]
                    nc.scalar.activation(
                        out=fh, in_=pss[t], func=Act.Identity,
                        scale=1.0 / (WS * WS))
                    nc.sync.dma_start(
                        out=out_d[b, t, :, h * HALF:(h + 1) * HALF], in_=fh)

        # --- software-pipelined emission across the two batches
        nc.sync.dma_start(out=smallc, in_=smallc_d[:, :])
        emit_x8loads(0)
        nc.sync.dma_start(out=gind, in_=gind_d[:, :])
        emit_x8loads(1)
        nc.sync.dma_start(out=wqk8, in_=wqk_d[:, :, :])
        nc.sync.dma_start(out=wv8, in_=wv_d[:, :, :])
        emit_stats_dve(0)
        mi0 = emit_gn_chain(0, 1.0)
        emit_gn_apply(0, mi0, ["act", "dve", "pool", "act"])
        emit_qkv(0, v_eng="dve")
        emit_xloads(0, as_r=True)
        nc.sync.dma_start(out=ow8, in_=ow_d[:, :, :])
        nc.sync.dma_start(out=r(id256), in_=r(id_d[:, :]))
        emit_stats_dve(1)
        emit_scores(0, [0, 1])
        mi1 = emit_gn_chain(1, 1.0)   # sqrt slots in after A(0,0..1)
        emit_scores(0, [2, 3, 4, 5, 6, 7])
        emit_gn_apply(1, mi1, ["dve", "act", "pool", "pool"])
        emit_qkv(1, v_eng="dve")     # PE filler while ACT drains A(0)
        emit_xloads(1, as_r=True)
        emit_rb(0)
        # interleave av(0) (DVE-drained) with scores(1) (ACT-drained) and
        # proj(0) so both eviction engines stay fed
        emit_av(0, [0, 1])
        emit_scores(1, [0, 1])
        emit_av(0, [2, 3])
        emit_scores(1, [2, 3])
        emit_scores(1, [4, 5])
        emit_scores(1, [6, 7])
        emit_rb(1)
        emit_proj(0, [0, 1, 2, 3], res_pe=True)
        emit_av(1, [0, 1, 2, 3], half_evict=True)
        emit_proj_halved(1)

    _split_multi_waits(nc)
    return nc


def _split_multi_waits(nc):
    """This neuronxcc walrus supports one sync-wait per ISA instruction.

    Tile emits instructions with several waits; hoist all but the last onto
    same-engine NoOps inserted immediately before (engine sequencers execute
    waits in order, so this is semantically identical).
    """
    from concourse import mybir

    n = 0
    for f in nc.m.functions:
        for bb in f.blocks:
            insts = bb.instructions
            out = []
            for inst in insts:
                si = inst.sync_info
                if si is not None and si.on_wait and len(si.on_wait) > 1:
                    waits = list(si.on_wait)
                    for w in waits[:-1]:
                        nop = mybir.InstNoOp(name=f"WSPLIT-{n}", ins=[], outs=[])
                        n += 1
                        nop.engine = inst.engine
                        nop.sync_info = mybir.SyncInfo(on_wait=[w], on_update=[])
                        out.append(nop)
                    inst.sync_info = mybir.SyncInfo(
                        on_wait=[waits[-1]], on_update=list(si.on_update or [])
                    )
                out.append(inst)
            if n:
                bb.instructions = out
    return nc


def _prep_consts(qkv_w, qkv_b, out_w, out_b, gn_w, gn_b):
    from ml_dtypes import float8_e4m3fn

    f = np.float32
    # lhsT chunk layouts: [p, t, o] = W.T[128t+p, o], scaled x16, fp8
    wqk = np.ascontiguousarray(
        (WS * qkv_w[:2 * C]).T.reshape(KC, 128, 2 * C).transpose(1, 0, 2)
    ).astype(float8_e4m3fn)
    wv = np.ascontiguousarray(
        (WS * qkv_w[2 * C:]).T.reshape(KC, 128, C).transpose(1, 0, 2)
    ).astype(float8_e4m3fn)
    ow = np.ascontiguousarray(
        (WS * out_w).T.reshape(KC, 128, C).transpose(1, 0, 2)
    ).astype(float8_e4m3fn)
    smallc = np.zeros((128, SC_COLS), dtype=f)
    for t in range(CT):
        cs = slice(128 * t, 128 * (t + 1))
        smallc[:, SC_GNW + t] = gn_w[cs]
        smallc[:, SC_GNB + t] = gn_b[cs]
    for mt in range(2 * CT):
        smallc[:, SC_QKB + mt] = WS * qkv_b[128 * mt:128 * (mt + 1)]
    beff = out_w @ qkv_b[2 * C:] + out_b
    with_beff = bool(np.any(beff != 0.0))
    gind = np.zeros((128, GI_COLS), dtype=f)
    for t in range(CT):
        for p_ in range(128):
            gind[p_, GI_GFWD + G * t + (128 * t + p_) // 16] = 1.0
            gind[(128 * t + p_) // 16, GI_GBWD + 128 * t + p_] = 1.0
    gind[0, GI_BEFFR:GI_BEFFR + C] = (WS * WS) * beff
    gind[0, GI_ONER:GI_ONER + HALF] = 1.0
    id256 = np.zeros((128, 128), dtype=f)
    id256[np.arange(128), np.arange(128)] = WS * WS
    return dict(wqk=wqk, wv=wv, ow=ow, smallc=smallc, gind=gind,
                id256=id256), with_beff


def kernel(x, gn_w, gn_b, qkv_w, qkv_b, out_w, out_b):
    from concourse.bass_utils import run_bass_kernel_spmd

    x = np.ascontiguousarray(np.asarray(x, dtype=np.float32))
    consts, with_beff = _prep_consts(
        np.asarray(qkv_w, np.float32), np.asarray(qkv_b, np.float32),
        np.asarray(out_w, np.float32), np.asarray(out_b, np.float32),
        np.asarray(gn_w, np.float32), np.asarray(gn_b, np.float32),
    )
    from ml_dtypes import bfloat16
    xr = x.reshape(NCORES, BPC, CT, 128, N)
    x8r = xr.astype(bfloat16)
    in_maps = [dict(x=np.ascontiguousarray(xr[i]), x8=np.ascontiguousarray(x8r[i]),
                    **consts) for i in range(NCORES)]

    key = ("nc", with_beff)
    if key not in _CACHE:
        _CACHE[key] = _build(with_beff)
    _CACHE["nc"] = _CACHE[key]
    res = run_bass_kernel_spmd(
        _CACHE[key], in_maps, core_ids=list(range(NCORES)),
        trace=_CACHE.get("trace", False),
    )
    _CACHE["last"] = res
    out = np.stack([r["out"] for r in res.results])  # [8, BPC, CT, 128, N]
    return out.reshape(B, C, 32, 32)
